# revision 23
# baseline (speedup 1.0000x reference)
"""Causal self-attention Trainium2 kernel (B=256, T=256, C=384, 8 heads x 48).

Strategy: pure data-parallel over batch across 8 NeuronCores (32 batches per
core, no collectives). All layouts are arranged on the host so the device
kernel never transposes anything:

  - x is sent transposed per batch: xT [nb, C, T].
  - QK projection computes q^T / k^T in "feature-major" layout [o', tokens]
    with heads padded to 64-row blocks, augmented with 2 extra contraction
    rows that carry the q/k bias cross terms, so scores come out exactly
    (up to a per-head constant, which softmax ignores).  K_contract = 50.
  - Scores are computed transposed, S^T[k, q], per head with row-tiled
    (tile_position) 2-head concurrency.  Causal structure is exploited:
    the fully-masked tile (k in [128,256), q in [0,128)) is never computed;
    the kx=1 score matmul covers only the upper q half.
  - Softmax skips the max-subtraction (inputs are well-scaled gaussians),
    exp on ACT straight PSUM->SBUF; only the two diagonal 128x128 tiles
    are masked (same 0/1 triangle for both) with a single small DVE mul.
  - The PV matmul consumes V in token-major layout (computed directly by
    swapping stationary/moving operands - no transpose), augmented with a
    leading ones column per head so row 0 of each head block is the softmax
    denominator. Col-tiled 2-head concurrency; kx=1 contributes only the
    upper q half.
  - Normalization: one custom-DVE approx-reciprocal op over both denominator
    rows (partitions 0 and 64 via a strided AP), partition-broadcast done by
    SBUF->SBUF DMA (stride-0 source), one fused multiply that also moves
    PSUM->SBUF.  ao row 0 of each head block becomes exactly 1.0, so the
    projection bias (incl. the folded V bias) rides in wp row 0 for free.
  - Output projection consumes the attention output directly in its
    [c', token] layout.

Matmuls run in bf16 (fp32 PSUM accumulation).
"""

import os
import sys

import numpy as np

try:
    import ml_dtypes

    BF16_NP = ml_dtypes.bfloat16
except ImportError:  # pragma: no cover
    BF16_NP = None

for _p in ("/opt/trn_rl_repo",):
    if os.path.isdir(_p) and _p not in sys.path:
        sys.path.insert(0, _p)

from contextlib import ExitStack

import concourse.bass as bass
import concourse.bacc as bacc
import concourse.tile as tile
from concourse import mybir
from concourse.bass_utils import run_bass_kernel_spmd

P = 128
T = 256
C = 384
NH = 8
HD = 48
KA = 50  # augmented contraction rows per head (48 + cq/ck row + ones row)
HB = 64  # padded head block stride
DT = mybir.dt.float32
BF = mybir.dt.bfloat16
AF = mybir.ActivationFunctionType
N_CORES = 8
B_FULL = 256
NB = B_FULL // N_CORES  # batches per core

MMDT = BF


def build_nc(nb: int = NB, debug: bool = False, repeat: int = 1, timing: bool = False):
    nc = bacc.Bacc(None)

    xT = nc.declare_dram_parameter("xT", [nb, C, T], MMDT, isOutput=False)
    wqk_d = nc.declare_dram_parameter("wqk", [3, P, 1024], MMDT, isOutput=False)
    wv_d = nc.declare_dram_parameter("wv", [3, P, 384], MMDT, isOutput=False)
    wp_d = nc.declare_dram_parameter("wp", [4, P, 384], MMDT, isOutput=False)
    bq_d = nc.declare_dram_parameter("bq", [P, 1], DT, isOutput=False)
    bk_d = nc.declare_dram_parameter("bk", [P, 1], DT, isOutput=False)
    tri4_d = nc.declare_dram_parameter("tri4", [P, 1024], BF, isOutput=False)
    vinit_d = nc.declare_dram_parameter("vinit", [P, 1024], BF, isOutput=False)
    if timing:
        y_d = nc.dram_tensor("y_int", [nb, T, C], DT)
        ydum_d = nc.declare_dram_parameter("ydum", [P, 1], DT, isOutput=True)
    else:
        y_d = nc.declare_dram_parameter("y", [nb, T, C], DT, isOutput=True)

    # Pin the ACT table set to the one covering Exp+Ln+Copy+Identity so the
    # table-load inserter never has to switch sets mid-kernel.
    from concourse.hw_specs import get_activation_tables

    act_sets = list(get_activation_tables(nc.m.arch).keys())
    combined_set_id = act_sets.index("natural_log_exp_and_others")

    with tile.TileContext(nc) as tc, ExitStack() as ctx:
        nc.scalar.add_instruction(
            mybir.InstLoadActFuncSet(
                name=nc.get_next_instruction_name(),
                act_func_set_id=combined_set_id,
                ins=[],
                outs=[],
            )
        )
        const = ctx.enter_context(tc.tile_pool(name="const", bufs=1))
        xtp = ctx.enter_context(tc.tile_pool(name="xt", bufs=6))
        qkp = ctx.enter_context(tc.tile_pool(name="qkt", bufs=2))
        vp = ctx.enter_context(tc.tile_pool(name="v", bufs=4))
        ptp = ctx.enter_context(tc.tile_pool(name="pt", bufs=2))
        aop = ctx.enter_context(tc.tile_pool(name="ao", bufs=2))
        yp = ctx.enter_context(tc.tile_pool(name="y", bufs=4))
        psA = ctx.enter_context(
            tc.tile_pool(name="psA", bufs=2, space=bass.MemorySpace.PSUM)
        )
        psS = ctx.enter_context(
            tc.tile_pool(name="psS", bufs=2, space=bass.MemorySpace.PSUM)
        )
        psO = ctx.enter_context(
            tc.tile_pool(name="psO", bufs=2, space=bass.MemorySpace.PSUM)
        )
        psW = ctx.enter_context(
            tc.tile_pool(name="psW", bufs=2, space=bass.MemorySpace.PSUM)
        )

        # ---- load constants ------------------------------------------------
        wqk_sb = []
        wv_sb = []
        wp_sb = []
        for ci in range(3):
            t = const.tile([P, 1024], MMDT, tag=f"wqk{ci}")
            nc.sync.dma_start(t[:], wqk_d[ci])
            wqk_sb.append(t)
        for ci in range(3):
            t = const.tile([P, 384], MMDT, tag=f"wv{ci}")
            nc.sync.dma_start(t[:], wv_d[ci])
            wv_sb.append(t)
        for cc in range(4):
            t = const.tile([P, 384], MMDT, tag=f"wp{cc}")
            nc.sync.dma_start(t[:], wp_d[cc])
            wp_sb.append(t)
        bq_sb = const.tile([P, 1], DT, tag="bq")
        nc.sync.dma_start(bq_sb[:], bq_d[:])
        bk_sb = const.tile([P, 1], DT, tag="bk")
        nc.sync.dma_start(bk_sb[:], bk_d[:])
        lnb_d = nc.declare_dram_parameter("lnb", [P, 1], DT, isOutput=False)
        lnb_sb = const.tile([P, 1], DT, tag="lnb")
        nc.sync.dma_start(lnb_sb[:], lnb_d[:])
        tri4_sb = const.tile([P, 1024], BF, tag="tri4")
        nc.sync.dma_start(tri4_sb[:], tri4_d[:])

        tri4_r = tri4_sb[:].rearrange("p (t s e) -> p t s e", s=2, e=128)

        # ---- per-batch-pair pipeline ---------------------------------------
        assert nb % 2 == 0
        for bp_it in range((nb // 2) * repeat):
            b0 = 2 * (bp_it % (nb // 2))
            xt = []
            for ci in range(3):
                t = xtp.tile([P, 2 * T], MMDT, tag="xt")
                nc.scalar.dma_start(
                    t[:].rearrange("p (b t) -> p b t", b=2),
                    xT[b0 : b0 + 2, 128 * ci : 128 * ci + 128, :].rearrange(
                        "b p t -> p b t"
                    ),
                )
                xt.append(t)

            # QK projection: 8 half-waves of [128, 512] psum, ping-ponged so
            # the copy of wave w overlaps the matmuls of wave w+1
            qk_sb = {}
            for w, name in ((0, "qt"), (1, "kt")):
                dst = qkp.tile([P, 2048], MMDT, tag=name)
                for oc in range(4):
                    ps = psA.tile([P, 512], DT, tag="psA")
                    for ci in range(3):
                        nc.tensor.matmul(
                            ps[:],
                            wqk_sb[ci][
                                :, 512 * w + 128 * oc : 512 * w + 128 * oc + 128
                            ],
                            xt[ci][:],
                            start=(ci == 0),
                            stop=(ci == 2),
                        )
                    bias = bq_sb if w == 0 else bk_sb
                    nc.vector.tensor_scalar_add(
                        dst[:, 512 * oc : 512 * oc + 512], ps[:], bias[:, 0:1]
                    )
                qk_sb[name] = dst
            qt, kt = qk_sb["qt"], qk_sb["kt"]

            # V in token-major layout (per batch), ones column via vinit DMA
            v_sbs = []
            for bb in range(2):
                vt = vp.tile([P, 1024], BF, tag="v")
                nc.sync.dma_start(vt[:], vinit_d[:])
                v_sbs.append(vt)
            for tch in range(4):
                bb, tcx = tch // 2, tch % 2
                psv = psW.tile([P, 384], DT, tag="ps384")
                for ci in range(3):
                    nc.tensor.matmul(
                        psv[:],
                        xt[ci][:, 256 * bb + 128 * tcx : 256 * bb + 128 * tcx + 128],
                        wv_sb[ci][:],
                        start=(ci == 0),
                        stop=(ci == 2),
                    )
                dst = v_sbs[bb][:, 512 * tcx : 512 * tcx + 512].rearrange(
                    "p (h c) -> p h c", c=HB
                )[:, :, 1 : 1 + HD]
                src = psv[:].rearrange("p (h c) -> p h c", c=HD)
                nc.vector.tensor_copy(dst, src)

            for bb in range(2):
                b = b0 + bb
                v_sb = v_sbs[bb]
                # S^T per head tile (g, j): [kx0: 256q | kx1: upper 128q].
                # Attention + normalization run per half-batch (2 head groups
                # at a time) so the norm chain of one half overlaps the PV of
                # the other and PSUM tiles stay one bank each.
                pt = ptp.tile([P, 3072], BF, tag="pt")
                ao = aop.tile([P, 1024], MMDT, tag="ao")
                for hg in range(2):
                    pso = psO.tile([P, 512], DT, tag="psO")
                    for g2 in range(2):
                        g = 2 * hg + g2
                        for j in range(2):
                            h = 2 * g + j
                            tix = 384 * h
                            base = HB * j
                            qc = 512 * g + 256 * bb
                            pss = psS.tile([P, 512], DT, tag="psS")
                            nc.tensor.matmul(
                                pss[:, 0:256],
                                kt[base : base + KA, qc : qc + 128],
                                qt[base : base + KA, qc : qc + 256],
                                start=True,
                                stop=True,
                            )
                            nc.tensor.matmul(
                                pss[:, 256:384],
                                kt[base : base + KA, qc + 128 : qc + 256],
                                qt[base : base + KA, qc + 128 : qc + 256],
                                start=True,
                                stop=True,
                            )
                            nc.scalar.activation(
                                pt[:, tix : tix + 384], pss[:, 0:384], AF.Exp
                            )

                    # mask all 4 head tiles of this half in one DVE op
                    pth = pt[:, 1536 * hg : 1536 * hg + 1536].rearrange(
                        "p (t s e) -> p t s e", s=3, e=128
                    )[:, :, 0::2, :]
                    nc.vector.tensor_mul(pth, pth, tri4_r)

                    # PV for the 4 heads of this half
                    for g2 in range(2):
                        g = 2 * hg + g2
                        for j in range(2):
                            h = 2 * g + j
                            tix = 384 * h
                            base = HB * j
                            nc.tensor.matmul(
                                pso[base : base + HB, 256 * g2 : 256 * g2 + 256],
                                v_sb[:, HB * h : HB * h + HB],
                                pt[:, tix : tix + 256],
                                start=True,
                                stop=False,
                                tile_position=(0, base),
                            )
                            nc.tensor.matmul(
                                pso[
                                    base : base + HB, 256 * g2 + 128 : 256 * g2 + 256
                                ],
                                v_sb[:, 512 + HB * h : 512 + HB * h + HB],
                                pt[:, tix + 256 : tix + 384],
                                start=False,
                                stop=True,
                                tile_position=(0, base),
                            )

                    # normalization: rec = exp(-ln(den)) on ACT (same table
                    # set as the softmax exp).  Rows 1..63 ride along (ACT
                    # cost scales with the free dim, not partitions); only
                    # rows 0 and 64 are read.  Row 64 hops to partition 0 via
                    # a DVE cross-partition copy so gpsimd can broadcast it.
                    lnd = aop.tile([P, 512], DT, tag="lnd")
                    rec = aop.tile([P, 512], BF, tag="rec")
                    rec_o = aop.tile([1, 512], BF, tag="reco")
                    nc.scalar.activation(
                        lnd[0:65, :], pso[0:65, :], AF.Ln, bias=lnb_sb[0:65, 0:1]
                    )
                    nc.scalar.activation(
                        rec[0:65, :], lnd[0:65, :], AF.Exp, scale=-1.0
                    )
                    nc.vector.tensor_copy(rec_o[0:1, :], rec[64:65, :])
                    denb = aop.tile([P, 512], BF, tag="denb")
                    nc.gpsimd.partition_broadcast(denb[:, :], rec_o[0:1, :])
                    nc.gpsimd.partition_broadcast(denb[0:64, :], rec[0:1, :])
                    nc.vector.tensor_mul(
                        ao[:, 512 * hg : 512 * hg + 512], pso[:], denb[:]
                    )

                # output projection (bias folded into wp row 0)
                for tcx in range(2):
                    psy = psW.tile([P, 384], DT, tag="ps384")
                    for cc in range(4):
                        nc.tensor.matmul(
                            psy[:],
                            ao[:, 256 * cc + 128 * tcx : 256 * cc + 128 * tcx + 128],
                            wp_sb[cc][:],
                            start=(cc == 0),
                            stop=(cc == 3),
                        )
                    ysb = yp.tile([P, 384], DT, tag="y")
                    nc.vector.tensor_copy(ysb[:], psy[:])
                    nc.sync.dma_start(
                        y_d[b, 128 * tcx : 128 * tcx + 128, :], ysb[:]
                    )

        if timing:
            nc.sync.dma_start(ydum_d[:], bq_sb[:])

    nc.compile()
    return nc


def make_consts(attn_w, attn_b, proj_w, proj_b):
    attn_w = np.asarray(attn_w, dtype=np.float32)
    attn_b = np.asarray(attn_b, dtype=np.float32)
    proj_w = np.asarray(proj_w, dtype=np.float32)
    proj_b = np.asarray(proj_b, dtype=np.float32)

    s = 1.0 / np.sqrt(HD)
    Wq, Wk, Wv = attn_w[0:C], attn_w[C : 2 * C], attn_w[2 * C : 3 * C]
    bq, bk, bv = attn_b[0:C], attn_b[C : 2 * C], attn_b[2 * C : 3 * C]

    # WQK: [C, 1024] -> [3, 128, 1024]
    M = np.zeros((C, 1024), dtype=np.float32)
    for h in range(NH):
        Wq_h = Wq[HD * h : HD * h + HD]  # [48, C]
        Wk_h = Wk[HD * h : HD * h + HD]
        bq_h = bq[HD * h : HD * h + HD]
        bk_h = bk[HD * h : HD * h + HD]
        # q-hat block
        M[:, HB * h : HB * h + HD] = (s * Wq_h).T
        M[:, HB * h + 48] = s * (bk_h @ Wq_h)  # c_q row
        # (row 49 of q-hat is the ones row via bias)
        # k-hat block
        M[:, 512 + HB * h : 512 + HB * h + HD] = Wk_h.T
        # (row 48 of k-hat is the ones row via bias)
        M[:, 512 + HB * h + 49] = s * (bq_h @ Wk_h)  # c_k row
    WQK = np.ascontiguousarray(M.reshape(C, 1024).reshape(3, P, 1024))

    # WV: packed [C, 384] -> [3, 128, 384]; col 48h+j = Wv row 48h+j
    WV = np.ascontiguousarray(Wv.T.reshape(3, P, 384))

    # WP: [512, 384] -> [4, 128, 384]; row HB*h + 1 + j = proj_w[:, HD*h+j]
    # row 0 carries the folded projection bias (ao row 0 == 1 after norm)
    bp_eff = proj_b + proj_w @ bv
    Wp_aug = np.zeros((512, C), dtype=np.float32)
    for h in range(NH):
        Wp_aug[HB * h + 1 : HB * h + 1 + HD, :] = proj_w[:, HD * h : HD * h + HD].T
    Wp_aug[0, :] = bp_eff
    WP = np.ascontiguousarray(Wp_aug.reshape(4, P, 384))

    BQ = np.zeros((P, 1), dtype=np.float32)
    BQ[49, 0] = 1.0
    BQ[49 + HB, 0] = 1.0
    BK = np.zeros((P, 1), dtype=np.float32)
    BK[48, 0] = 1.0
    BK[48 + HB, 0] = 1.0

    # causal 0/1 mask for the two diagonal S^T[k, q] tiles (identical)
    kk = np.arange(128)[:, None]
    qq = np.arange(128)[None, :]
    tri = (qq >= kk).astype(np.float32)  # [128k, 128q]
    TRI4 = np.ascontiguousarray(np.concatenate([tri, tri] * 4, axis=1)).astype(BF16_NP)

    # ln bias: 0 on the real den rows (0, 64), large elsewhere so the
    # throwaway rows stay finite through ln
    LNB = np.full((P, 1), 1e6, dtype=np.float32)
    LNB[0, 0] = 0.0
    LNB[64, 0] = 0.0

    # v-init pattern: ones column at 64h of each kx half, zeros elsewhere
    vinit_row = np.zeros(1024, dtype=np.float32)
    for kx in range(2):
        for h in range(NH):
            vinit_row[512 * kx + HB * h] = 1.0
    VINIT = np.ascontiguousarray(np.broadcast_to(vinit_row[None, :], (P, 1024))).astype(
        BF16_NP
    )

    WQK = WQK.astype(BF16_NP)
    WV = WV.astype(BF16_NP)
    WP = WP.astype(BF16_NP)

    return {
        "vinit": VINIT,
        "wqk": WQK,
        "wv": WV,
        "wp": WP,
        "bq": BQ,
        "bk": BK,
        "lnb": LNB,
        "tri4": TRI4,
    }


_NC_CACHE = {}


def get_nc(nb: int = NB):
    if nb not in _NC_CACHE:
        _NC_CACHE[nb] = build_nc(nb)
    return _NC_CACHE[nb]


def make_in_maps(x, attn_w, attn_b, proj_w, proj_b):
    x = np.asarray(x, dtype=np.float32)
    consts = make_consts(attn_w, attn_b, proj_w, proj_b)
    in_maps = []
    for core in range(N_CORES):
        xs = x[core * NB : (core + 1) * NB]  # [NB, T, C]
        xTl = np.ascontiguousarray(xs.transpose(0, 2, 1)).astype(BF16_NP)
        m = {"xT": xTl}
        m.update(consts)
        in_maps.append(m)
    return in_maps


def kernel(x, attn_w, attn_b, proj_w, proj_b):
    nc = get_nc(NB)
    in_maps = make_in_maps(x, attn_w, attn_b, proj_w, proj_b)
    res = run_bass_kernel_spmd(nc, in_maps, core_ids=list(range(N_CORES)))
    out = np.concatenate(
        [res.results[i]["y"] for i in range(N_CORES)], axis=0
    ).astype(np.float32)
    return out


# revision 25
# speedup vs baseline: 1.1689x; 1.1689x over previous
"""Causal self-attention Trainium2 kernel (B=256, T=256, C=384, 8 heads x 48).

Strategy: pure data-parallel over batch across 8 NeuronCores (32 batches per
core, no collectives). All layouts are arranged on the host so the device
kernel never transposes anything:

  - x is sent transposed per batch: xT [nb, C, T].
  - QK projection computes q^T / k^T in "feature-major" layout [o', tokens]
    with heads padded to 64-row blocks, augmented with 2 extra contraction
    rows that carry the q/k bias cross terms, so scores come out exactly
    (up to a per-head constant, which softmax ignores).  K_contract = 50.
  - Scores are computed transposed, S^T[k, q], per head with row-tiled
    (tile_position) 2-head concurrency.  Causal structure is exploited:
    the fully-masked tile (k in [128,256), q in [0,128)) is never computed;
    the kx=1 score matmul covers only the upper q half.
  - Softmax skips the max-subtraction (inputs are well-scaled gaussians),
    exp on ACT straight PSUM->SBUF; only the two diagonal 128x128 tiles
    are masked (same 0/1 triangle for both) with a single small DVE mul.
  - The PV matmul consumes V in token-major layout (computed directly by
    swapping stationary/moving operands - no transpose), augmented with a
    leading ones column per head so row 0 of each head block is the softmax
    denominator. Col-tiled 2-head concurrency; kx=1 contributes only the
    upper q half.
  - Normalization: one custom-DVE approx-reciprocal op over both denominator
    rows (partitions 0 and 64 via a strided AP), partition-broadcast done by
    SBUF->SBUF DMA (stride-0 source), one fused multiply that also moves
    PSUM->SBUF.  ao row 0 of each head block becomes exactly 1.0, so the
    projection bias (incl. the folded V bias) rides in wp row 0 for free.
  - Output projection consumes the attention output directly in its
    [c', token] layout.

Matmuls run in bf16 (fp32 PSUM accumulation).
"""

import os
import sys

import numpy as np

try:
    import ml_dtypes

    BF16_NP = ml_dtypes.bfloat16
except ImportError:  # pragma: no cover
    BF16_NP = None

for _p in ("/opt/trn_rl_repo",):
    if os.path.isdir(_p) and _p not in sys.path:
        sys.path.insert(0, _p)

from contextlib import ExitStack

import concourse.bass as bass
import concourse.bacc as bacc
import concourse.tile as tile
from concourse import mybir
from concourse.bass_utils import run_bass_kernel_spmd

P = 128
T = 256
C = 384
NH = 8
HD = 48
KA = 50  # augmented contraction rows per head (48 + cq/ck row + ones row)
HB = 64  # padded head block stride
DT = mybir.dt.float32
BF = mybir.dt.bfloat16
AF = mybir.ActivationFunctionType
N_CORES = 8
B_FULL = 256
NB = B_FULL // N_CORES  # batches per core

MMDT = BF


def build_nc(nb: int = NB, debug: bool = False, repeat: int = 1, timing: bool = False):
    nc = bacc.Bacc(None)

    xT = nc.declare_dram_parameter("xT", [nb, C, T], MMDT, isOutput=False)
    wqk_d = nc.declare_dram_parameter("wqk", [3, P, 1024], MMDT, isOutput=False)
    wv_d = nc.declare_dram_parameter("wv", [3, P, 384], MMDT, isOutput=False)
    wp_d = nc.declare_dram_parameter("wp", [4, P, 384], MMDT, isOutput=False)
    bq_d = nc.declare_dram_parameter("bq", [P, 1], DT, isOutput=False)
    bk_d = nc.declare_dram_parameter("bk", [P, 1], DT, isOutput=False)
    tri4_d = nc.declare_dram_parameter("tri4", [P, 1024], BF, isOutput=False)
    vinit_d = nc.declare_dram_parameter("vinit", [P, 1024], BF, isOutput=False)
    if timing:
        y_d = nc.dram_tensor("y_int", [nb, T, C], DT)
        ydum_d = nc.declare_dram_parameter("ydum", [P, 1], DT, isOutput=True)
    else:
        y_d = nc.declare_dram_parameter("y", [nb, T, C], DT, isOutput=True)

    # Pin the ACT table set to the one covering Exp+Ln+Copy+Identity so the
    # table-load inserter never has to switch sets mid-kernel.
    from concourse.hw_specs import get_activation_tables

    act_sets = list(get_activation_tables(nc.m.arch).keys())
    combined_set_id = act_sets.index("natural_log_exp_and_others")

    with tile.TileContext(nc) as tc, ExitStack() as ctx:
        nc.scalar.add_instruction(
            mybir.InstLoadActFuncSet(
                name=nc.get_next_instruction_name(),
                act_func_set_id=combined_set_id,
                ins=[],
                outs=[],
            )
        )
        const = ctx.enter_context(tc.tile_pool(name="const", bufs=1))
        xtp = ctx.enter_context(tc.tile_pool(name="xt", bufs=6))
        qkp = ctx.enter_context(tc.tile_pool(name="qkt", bufs=2))
        vp = ctx.enter_context(tc.tile_pool(name="v", bufs=4))
        ptp = ctx.enter_context(tc.tile_pool(name="pt", bufs=2))
        aop = ctx.enter_context(tc.tile_pool(name="ao", bufs=2))
        yp = ctx.enter_context(tc.tile_pool(name="y", bufs=4))
        psA = ctx.enter_context(
            tc.tile_pool(name="psA", bufs=2, space=bass.MemorySpace.PSUM)
        )
        psS = ctx.enter_context(
            tc.tile_pool(name="psS", bufs=2, space=bass.MemorySpace.PSUM)
        )
        psO = ctx.enter_context(
            tc.tile_pool(name="psO", bufs=2, space=bass.MemorySpace.PSUM)
        )
        psW = ctx.enter_context(
            tc.tile_pool(name="psW", bufs=2, space=bass.MemorySpace.PSUM)
        )

        # ---- load constants ------------------------------------------------
        wqk_sb = []
        wv_sb = []
        wp_sb = []
        for ci in range(3):
            t = const.tile([P, 1024], MMDT, tag=f"wqk{ci}")
            nc.sync.dma_start(t[:], wqk_d[ci])
            wqk_sb.append(t)
        for ci in range(3):
            t = const.tile([P, 384], MMDT, tag=f"wv{ci}")
            nc.sync.dma_start(t[:], wv_d[ci])
            wv_sb.append(t)
        for cc in range(4):
            t = const.tile([P, 384], MMDT, tag=f"wp{cc}")
            nc.sync.dma_start(t[:], wp_d[cc])
            wp_sb.append(t)
        bq_sb = const.tile([P, 1], DT, tag="bq")
        nc.sync.dma_start(bq_sb[:], bq_d[:])
        bk_sb = const.tile([P, 1], DT, tag="bk")
        nc.sync.dma_start(bk_sb[:], bk_d[:])
        lnb_d = nc.declare_dram_parameter("lnb", [P, 1], DT, isOutput=False)
        lnb_sb = const.tile([P, 1], DT, tag="lnb")
        nc.sync.dma_start(lnb_sb[:], lnb_d[:])
        tri4_sb = const.tile([P, 1024], BF, tag="tri4")
        nc.sync.dma_start(tri4_sb[:], tri4_d[:])

        tri4_r = tri4_sb[:].rearrange("p (t s e) -> p t s e", s=2, e=128)

        # ---- per-batch-pair pipeline (software-pipelined: QKV/V of pair
        # p+1 is emitted before the projection of pair p so the PE can fill
        # the normalization-chain latency with projection-independent work)
        assert nb % 2 == 0

        def emit_qkv_v(b0):
            xt = []
            for ci in range(3):
                t = xtp.tile([P, 2 * T], MMDT, tag="xt", name=f"xt{b0}_{ci}")
                nc.sync.dma_start(
                    t[:].rearrange("p (b t) -> p b t", b=2),
                    xT[b0 : b0 + 2, 128 * ci : 128 * ci + 128, :].rearrange(
                        "b p t -> p b t"
                    ),
                )
                xt.append(t)

            # QK projection: 8 half-waves of [128, 512] psum, ping-ponged so
            # the copy of wave w overlaps the matmuls of wave w+1
            qk_sb = {}
            for w, name in ((0, "qt"), (1, "kt")):
                dst = qkp.tile([P, 2048], MMDT, tag=name, name=f"{name}{b0}")
                for oc in range(4):
                    ps = psA.tile([P, 512], DT, tag="psA", name=f"psA{b0}_{w}{oc}")
                    for ci in range(3):
                        nc.tensor.matmul(
                            ps[:],
                            wqk_sb[ci][
                                :, 512 * w + 128 * oc : 512 * w + 128 * oc + 128
                            ],
                            xt[ci][:],
                            start=(ci == 0),
                            stop=(ci == 2),
                        )
                    bias = bq_sb if w == 0 else bk_sb
                    nc.vector.tensor_scalar_add(
                        dst[:, 512 * oc : 512 * oc + 512], ps[:], bias[:, 0:1]
                    )
                qk_sb[name] = dst

            # V in token-major layout (per batch), ones column via vinit DMA
            v_sbs = []
            for bb in range(2):
                vt = vp.tile([P, 1024], BF, tag="v", name=f"v{b0}_{bb}")
                nc.sync.dma_start(vt[:], vinit_d[:])
                v_sbs.append(vt)
            for tch in range(4):
                bb, tcx = tch // 2, tch % 2
                psv = psW.tile([P, 384], DT, tag="ps384", name=f"psv{b0}_{tch}")
                for ci in range(3):
                    nc.tensor.matmul(
                        psv[:],
                        xt[ci][:, 256 * bb + 128 * tcx : 256 * bb + 128 * tcx + 128],
                        wv_sb[ci][:],
                        start=(ci == 0),
                        stop=(ci == 2),
                    )
                dst = v_sbs[bb][:, 512 * tcx : 512 * tcx + 512].rearrange(
                    "p (h c) -> p h c", c=HB
                )[:, :, 1 : 1 + HD]
                src = psv[:].rearrange("p (h c) -> p h c", c=HD)
                nc.vector.tensor_copy(dst, src)
            return qk_sb["qt"], qk_sb["kt"], v_sbs

        n_it = (nb // 2) * repeat
        cur = emit_qkv_v(0)
        for bp_it in range(n_it):
            b0 = 2 * (bp_it % (nb // 2))
            qt, kt, v_sbs = cur
            aos = []
            for bb in range(2):
                b = b0 + bb
                v_sb = v_sbs[bb]
                # S^T per head tile (g, j): [kx0: 256q | kx1: upper 128q].
                # Attention + normalization run per half-batch (2 head groups
                # at a time) so the norm chain of one half overlaps the PV of
                # the other and PSUM tiles stay one bank each.
                pt = ptp.tile([P, 3072], BF, tag="pt")
                ao = aop.tile([P, 1024], MMDT, tag="ao")
                for hg in range(2):
                    pso = psO.tile([P, 512], DT, tag="psO")
                    for g2 in range(2):
                        g = 2 * hg + g2
                        for j in range(2):
                            h = 2 * g + j
                            tix = 384 * h
                            base = HB * j
                            qc = 512 * g + 256 * bb
                            pss = psS.tile([P, 512], DT, tag="psS")
                            nc.tensor.matmul(
                                pss[:, 0:256],
                                kt[base : base + KA, qc : qc + 128],
                                qt[base : base + KA, qc : qc + 256],
                                start=True,
                                stop=True,
                            )
                            nc.tensor.matmul(
                                pss[:, 256:384],
                                kt[base : base + KA, qc + 128 : qc + 256],
                                qt[base : base + KA, qc + 128 : qc + 256],
                                start=True,
                                stop=True,
                            )
                            nc.scalar.activation(
                                pt[:, tix : tix + 384], pss[:, 0:384], AF.Exp
                            )

                    # mask all 4 head tiles of this half in one DVE op
                    pth = pt[:, 1536 * hg : 1536 * hg + 1536].rearrange(
                        "p (t s e) -> p t s e", s=3, e=128
                    )[:, :, 0::2, :]
                    nc.vector.tensor_mul(pth, pth, tri4_r)

                    # PV for the 4 heads of this half
                    for g2 in range(2):
                        g = 2 * hg + g2
                        for j in range(2):
                            h = 2 * g + j
                            tix = 384 * h
                            base = HB * j
                            nc.tensor.matmul(
                                pso[base : base + HB, 256 * g2 : 256 * g2 + 256],
                                v_sb[:, HB * h : HB * h + HB],
                                pt[:, tix : tix + 256],
                                start=True,
                                stop=False,
                                tile_position=(0, base),
                            )
                            nc.tensor.matmul(
                                pso[
                                    base : base + HB, 256 * g2 + 128 : 256 * g2 + 256
                                ],
                                v_sb[:, 512 + HB * h : 512 + HB * h + HB],
                                pt[:, tix + 256 : tix + 384],
                                start=False,
                                stop=True,
                                tile_position=(0, base),
                            )

                    # normalization: rec = exp(-ln(den)) on ACT (same table
                    # set as the softmax exp).  Rows 1..63 ride along (ACT
                    # cost scales with the free dim, not partitions); only
                    # rows 0 and 64 are read.  Row 64 hops to partition 0 via
                    # a DVE cross-partition copy so gpsimd can broadcast it.
                    lnd = aop.tile([P, 512], DT, tag="lnd")
                    rec = aop.tile([P, 512], BF, tag="rec")
                    rec_o = aop.tile([1, 512], BF, tag="reco")
                    nc.scalar.activation(
                        lnd[0:65, :], pso[0:65, :], AF.Ln, bias=lnb_sb[0:65, 0:1]
                    )
                    nc.scalar.activation(
                        rec[0:65, :], lnd[0:65, :], AF.Exp, scale=-1.0
                    )
                    nc.vector.tensor_copy(rec_o[0:1, :], rec[64:65, :])
                    denb = aop.tile([P, 512], BF, tag="denb")
                    nc.gpsimd.partition_broadcast(denb[:, :], rec_o[0:1, :])
                    nc.gpsimd.partition_broadcast(denb[0:64, :], rec[0:1, :])
                    nc.vector.tensor_mul(
                        ao[:, 512 * hg : 512 * hg + 512], pso[:], denb[:]
                    )
                aos.append(ao)

            # prefetch + project the NEXT pair before this pair's projection
            if bp_it + 1 < n_it:
                cur = emit_qkv_v(2 * ((bp_it + 1) % (nb // 2)))

            # output projection (bias folded into wp row 0)
            for bb in range(2):
                b = b0 + bb
                ao = aos[bb]
                for tcx in range(2):
                    psy = psW.tile([P, 384], DT, tag="ps384", name=f"psy{b}_{tcx}")
                    for cc in range(4):
                        nc.tensor.matmul(
                            psy[:],
                            ao[:, 256 * cc + 128 * tcx : 256 * cc + 128 * tcx + 128],
                            wp_sb[cc][:],
                            start=(cc == 0),
                            stop=(cc == 3),
                        )
                    ysb = yp.tile([P, 384], DT, tag="y", name=f"y{b}_{tcx}")
                    nc.vector.tensor_copy(ysb[:], psy[:])
                    nc.sync.dma_start(
                        y_d[b, 128 * tcx : 128 * tcx + 128, :], ysb[:]
                    )

        if timing:
            nc.sync.dma_start(ydum_d[:], bq_sb[:])

    nc.compile()
    return nc


def make_consts(attn_w, attn_b, proj_w, proj_b):
    attn_w = np.asarray(attn_w, dtype=np.float32)
    attn_b = np.asarray(attn_b, dtype=np.float32)
    proj_w = np.asarray(proj_w, dtype=np.float32)
    proj_b = np.asarray(proj_b, dtype=np.float32)

    s = 1.0 / np.sqrt(HD)
    Wq, Wk, Wv = attn_w[0:C], attn_w[C : 2 * C], attn_w[2 * C : 3 * C]
    bq, bk, bv = attn_b[0:C], attn_b[C : 2 * C], attn_b[2 * C : 3 * C]

    # WQK: [C, 1024] -> [3, 128, 1024]
    M = np.zeros((C, 1024), dtype=np.float32)
    for h in range(NH):
        Wq_h = Wq[HD * h : HD * h + HD]  # [48, C]
        Wk_h = Wk[HD * h : HD * h + HD]
        bq_h = bq[HD * h : HD * h + HD]
        bk_h = bk[HD * h : HD * h + HD]
        # q-hat block
        M[:, HB * h : HB * h + HD] = (s * Wq_h).T
        M[:, HB * h + 48] = s * (bk_h @ Wq_h)  # c_q row
        # (row 49 of q-hat is the ones row via bias)
        # k-hat block
        M[:, 512 + HB * h : 512 + HB * h + HD] = Wk_h.T
        # (row 48 of k-hat is the ones row via bias)
        M[:, 512 + HB * h + 49] = s * (bq_h @ Wk_h)  # c_k row
    WQK = np.ascontiguousarray(M.reshape(C, 1024).reshape(3, P, 1024))

    # WV: packed [C, 384] -> [3, 128, 384]; col 48h+j = Wv row 48h+j
    WV = np.ascontiguousarray(Wv.T.reshape(3, P, 384))

    # WP: [512, 384] -> [4, 128, 384]; row HB*h + 1 + j = proj_w[:, HD*h+j]
    # row 0 carries the folded projection bias (ao row 0 == 1 after norm)
    bp_eff = proj_b + proj_w @ bv
    Wp_aug = np.zeros((512, C), dtype=np.float32)
    for h in range(NH):
        Wp_aug[HB * h + 1 : HB * h + 1 + HD, :] = proj_w[:, HD * h : HD * h + HD].T
    Wp_aug[0, :] = bp_eff
    WP = np.ascontiguousarray(Wp_aug.reshape(4, P, 384))

    BQ = np.zeros((P, 1), dtype=np.float32)
    BQ[49, 0] = 1.0
    BQ[49 + HB, 0] = 1.0
    BK = np.zeros((P, 1), dtype=np.float32)
    BK[48, 0] = 1.0
    BK[48 + HB, 0] = 1.0

    # causal 0/1 mask for the two diagonal S^T[k, q] tiles (identical)
    kk = np.arange(128)[:, None]
    qq = np.arange(128)[None, :]
    tri = (qq >= kk).astype(np.float32)  # [128k, 128q]
    TRI4 = np.ascontiguousarray(np.concatenate([tri, tri] * 4, axis=1)).astype(BF16_NP)

    # ln bias: 0 on the real den rows (0, 64), large elsewhere so the
    # throwaway rows stay finite through ln
    LNB = np.full((P, 1), 1e6, dtype=np.float32)
    LNB[0, 0] = 0.0
    LNB[64, 0] = 0.0

    # v-init pattern: ones column at 64h of each kx half, zeros elsewhere
    vinit_row = np.zeros(1024, dtype=np.float32)
    for kx in range(2):
        for h in range(NH):
            vinit_row[512 * kx + HB * h] = 1.0
    VINIT = np.ascontiguousarray(np.broadcast_to(vinit_row[None, :], (P, 1024))).astype(
        BF16_NP
    )

    WQK = WQK.astype(BF16_NP)
    WV = WV.astype(BF16_NP)
    WP = WP.astype(BF16_NP)

    return {
        "vinit": VINIT,
        "wqk": WQK,
        "wv": WV,
        "wp": WP,
        "bq": BQ,
        "bk": BK,
        "lnb": LNB,
        "tri4": TRI4,
    }


_NC_CACHE = {}


def get_nc(nb: int = NB):
    if nb not in _NC_CACHE:
        _NC_CACHE[nb] = build_nc(nb)
    return _NC_CACHE[nb]


def make_in_maps(x, attn_w, attn_b, proj_w, proj_b):
    x = np.asarray(x, dtype=np.float32)
    consts = make_consts(attn_w, attn_b, proj_w, proj_b)
    in_maps = []
    for core in range(N_CORES):
        xs = x[core * NB : (core + 1) * NB]  # [NB, T, C]
        xTl = np.ascontiguousarray(xs.transpose(0, 2, 1)).astype(BF16_NP)
        m = {"xT": xTl}
        m.update(consts)
        in_maps.append(m)
    return in_maps


def kernel(x, attn_w, attn_b, proj_w, proj_b):
    nc = get_nc(NB)
    in_maps = make_in_maps(x, attn_w, attn_b, proj_w, proj_b)
    res = run_bass_kernel_spmd(nc, in_maps, core_ids=list(range(N_CORES)))
    out = np.concatenate(
        [res.results[i]["y"] for i in range(N_CORES)], axis=0
    ).astype(np.float32)
    return out
